# revision 10
# baseline (speedup 1.0000x reference)
"""ConformerAttention Trainium2 kernel, v4 (algebraic restructure, pipelined).

Per head h: scores_h = x M_h x^T + pos with M_h = Wq_h Wk_h^T / 8 (41x41,
host-precomputed), and out = sum_h (A_h x) G_h with G_h = Wv_h Wo_h
(41x1024, host-precomputed). Q/K/V projections and the 1024-wide AV
matmul disappear from the device.

Heads are processed in pairs padded to 64-partition slots (rows 0:41 and
64:105 — engine access patterns require 32-aligned partition starts).
Per-head matmuls write the two slots of a pair PSUM tile directly
(out/ifmap base partition 64 is a legal PE tile position), with lhsT
taken straight from column slices of xT / x-natural tiles; shifted
copies at partitions 64:105 serve the odd-head slot. Per block of NB=8
batches (W = 328 cols):
  xT      = transpose(x_block)  (bf16, 3 matmuls)
  x-nat   = one strided DMA (b t) d -> t (b d)
  yT_p    = blockdiag(M_2p, M_2p+1)-contract    8 matmuls N=328
  scoresT = pos-init (lhsT=[I|..|I]) + 128 accumulating matmuls N=41
  E = exp(scoresT) (scalar); Z via slot-ones lhsT: 8 matmuls N=328
  BT      = x_b^T @ E per head slot             128 matmuls N=41
  BTn     = BT * (1/Z)  (vector) -> 8 pair tiles [105, W]
  Y       = sum_p BTn_p^T @ G_p (K=105)         48 matmuls N=512

The pair loop is software-pipelined (yT[p] | scores[p-1] | Z[p-2] |
BT[p-3]) so the tensor queue never waits on a PSUM evacuation, and block
k+1's x load/transposes are emitted before Y(k) to hide prologues.

Data parallel over batch: 8 cores x 256 batches. Output stored bf16.
"""

import math
import sys

import numpy as np

sys.path.insert(0, "/opt/trn_rl_repo")

import concourse.bass as bass
import concourse.bacc as bacc
import concourse.mybir as mybir
from concourse import tile
from concourse.bass_utils import run_bass_kernel_spmd

F32 = mybir.dt.float32
BF16 = mybir.dt.bfloat16

B, S, DIN = 2048, 41, 41
U, H, DH = 1024, 16, 64
NC = 8
BC = B // NC          # 256 batches per core
NB = 8                # batches per block
NBLK = BC // NB       # 32 blocks
W = NB * S            # 328 free-dim columns per block
NP = 8                # head pairs
SLOT = 64             # partition slot per head (32-aligned starts)
KP = SLOT + S         # 105 used partitions of a pair tile
ROWS3 = (124, 124, 80)  # x rows per transpose (even: 4B-aligned bf16 PSUM cols)
YGRP = ((0, 123), (123, 123), (246, 82))  # (row0, nrows) of Y row-groups


def build_kernel(nc: bass.Bass, nblk: int = NBLK):
    x_d = nc.declare_dram_parameter("x", [BC * S, DIN], BF16, isOutput=False)
    m_d = nc.declare_dram_parameter("mcat", [DIN, NP * 128], BF16, isOutput=False)
    g_d = nc.declare_dram_parameter("gcat", [NP * 128, U], BF16, isOutput=False)
    pos_d = nc.declare_dram_parameter("pos_patT", [S, W], BF16, isOutput=False)
    posl_d = nc.declare_dram_parameter("posl", [S, 128], BF16, isOutput=False)
    od_d = nc.declare_dram_parameter("onesdiag", [KP, 128], BF16, isOutput=False)
    idb_d = nc.declare_dram_parameter("identb", [128, 128], BF16, isOutput=False)
    out_d = nc.declare_dram_parameter("out", [BC * S, U], BF16, isOutput=True)

    from contextlib import ExitStack
    with tile.TileContext(nc) as tc, ExitStack() as st:
        cpool = st.enter_context(tc.tile_pool(name="consts", bufs=1))
        ps = st.enter_context(tc.tile_pool(name="ps", bufs=8, space="PSUM"))
        xinp = st.enter_context(tc.tile_pool(name="xin", bufs=6))
        xtp = st.enter_context(tc.tile_pool(name="xt", bufs=2))
        xhp = st.enter_context(tc.tile_pool(name="xhi", bufs=4))
        ytp = st.enter_context(tc.tile_pool(name="yt", bufs=6))
        ep = st.enter_context(tc.tile_pool(name="E", bufs=10))
        rzp = st.enter_context(tc.tile_pool(name="rz", bufs=6))
        btp = st.enter_context(tc.tile_pool(name="btn", bufs=16))
        yp = st.enter_context(tc.tile_pool(name="y", bufs=6))

        # ---- constants ----
        mcat = cpool.tile([DIN, NP * 128], BF16, tag="mcat")
        nc.sync.dma_start(mcat[:], m_d[:])
        gch = []
        for p in range(NP):
            t = cpool.tile([KP, U], BF16, tag=f"g{p}")
            nc.sync.dma_start(t[:], g_d[p * 128:p * 128 + KP, :])
            gch.append(t)
        onesd = cpool.tile([KP, 128], BF16, tag="onesd")
        nc.sync.dma_start(onesd[:], od_d[:])
        posp = cpool.tile([S, W], BF16, tag="posp")
        nc.sync.dma_start(posp[:], pos_d[:])
        posl = cpool.tile([S, 128], BF16, tag="posl")
        nc.sync.dma_start(posl[:], posl_d[:])
        idb = cpool.tile([128, 128], BF16, tag="idb")
        nc.sync.dma_start(idb[:], idb_d[:])

        def stage(blk):
            """Load x for block blk, build xT / x-nat and shifted copies."""
            row0 = blk * NB * S
            tp = ps.tile([S, W], BF16, tag="ps", name="tp")
            off = 0
            for rows in ROWS3:
                xin = xinp.tile([128, DIN], BF16, tag="xin", name="xin")
                nc.sync.dma_start(xin[:rows, :], x_d[row0 + off:row0 + off + rows, :])
                nc.tensor.transpose(tp[:, off:off + rows], xin[:rows, :],
                                    idb[:rows, :rows])
                off += rows
            xt = xtp.tile([S, W], BF16, tag="xt", name="xt")
            nc.vector.tensor_copy(xt[:], tp[:])
            xnat = xtp.tile([S, W], BF16, tag="xnat", name="xnat")
            nc.sync.dma_start(
                xnat[:].rearrange("t (b d) -> t b d", b=NB),
                x_d[row0:row0 + NB * S, :].rearrange("(b t) d -> t b d", t=S))
            xth = xhp.tile([KP, W], BF16, tag="xth", name="xth")
            nc.gpsimd.tensor_copy(xth[SLOT:KP, :], xt[:])
            xnh = xhp.tile([KP, W], BF16, tag="xnh", name="xnh")
            nc.gpsimd.tensor_copy(xnh[SLOT:KP, :], xnat[:])
            return xt, xnat, xth, xnh

        def emit_yt(xt, p, yts):
            yps = ps.tile([KP, W], F32, tag="ps", name="yps")
            nc.tensor.matmul(yps[:], mcat[:, p * 128:p * 128 + KP], xt[:])
            yt = ytp.tile([KP, W], BF16, tag="yt", name="yt")
            nc.scalar.copy(yt[:], yps[:])
            yts[p] = yt

        def emit_scores(xs, p, yts, Es):
            xt, xnat, xth, xnh = xs
            sps = ps.tile([128, W], F32, tag="ps", name="sps")
            nc.tensor.matmul(sps[:], posl[:], posp[:], start=True, stop=False)
            yt = yts[p]
            for b in range(NB):
                c = slice(b * S, (b + 1) * S)
                nc.tensor.matmul(sps[0:S, c], xt[:, c], yt[0:S, c],
                                 start=False, stop=False)
                nc.tensor.matmul(sps[SLOT:KP, c], xth[SLOT:KP, c],
                                 yt[SLOT:KP, c],
                                 start=False, stop=(b == NB - 1))
            e = ep.tile([KP, W], BF16, tag="E", name="e")
            nc.scalar.activation(e[:], sps[:KP, :],
                                 mybir.ActivationFunctionType.Exp)
            Es[p] = e

        def emit_z(p, Es, rzs):
            zps = ps.tile([128, W], F32, tag="ps", name="zps")
            nc.tensor.matmul(zps[:], onesd[:], Es[p][:])
            r = rzp.tile([KP, W], F32, tag="rz", name="r")
            nc.vector.reciprocal_approx_fast(r[:], zps[:KP, :])
            rzs[p] = r

        def emit_bt(xs, p, Es, rzs, btns):
            xt, xnat, xth, xnh = xs
            bt = ps.tile([128, W], F32, tag="ps", name="bt")
            e = Es[p]
            for b in range(NB):
                c = slice(b * S, (b + 1) * S)
                nc.tensor.matmul(bt[0:S, c], xnat[:, c], e[0:S, c])
                nc.tensor.matmul(bt[SLOT:KP, c], xnh[SLOT:KP, c],
                                 e[SLOT:KP, c])
            bn = btp.tile([KP, W], BF16, tag="btn", name="bn")
            nc.vector.tensor_mul(bn[:], bt[:KP, :], rzs[p][:])
            btns[p] = bn

        xs_cur = stage(0)
        for blk in range(nblk):
            row0 = blk * NB * S

            yts, Es, rzs, btns = {}, {}, {}, {}
            for p in range(NP + 3):
                if p < NP:
                    emit_yt(xs_cur[0], p, yts)
                if 1 <= p <= NP:
                    emit_scores(xs_cur, p - 1, yts, Es)
                if 2 <= p <= NP + 1:
                    emit_z(p - 2, Es, rzs)
                if 3 <= p <= NP + 2:
                    emit_bt(xs_cur, p - 3, Es, rzs, btns)

            # next block's loads/copies overlap with this block's Y phase
            xs_next = stage(blk + 1) if blk + 1 < nblk else None

            for gi, (r0g, rows) in enumerate(YGRP):
                y = yp.tile([128, U], BF16, tag="y", name="y")
                for half in range(2):
                    yps2 = ps.tile([128, 512], F32, tag="ps", name="yps2")
                    for p in range(NP):
                        nc.tensor.matmul(
                            yps2[:rows, :],
                            btns[p][:, r0g:r0g + rows],
                            gch[p][:, half * 512:(half + 1) * 512],
                            start=(p == 0), stop=(p == NP - 1))
                    if half == 0:
                        nc.scalar.copy(y[:rows, :512], yps2[:rows, :])
                    else:
                        nc.vector.tensor_copy(y[:rows, 512:], yps2[:rows, :])
                nc.sync.dma_start(
                    out_d[row0 + r0g:row0 + r0g + rows, :], y[:rows, :])

            if xs_next is not None:
                xs_cur = xs_next

    return nc


_NC_CACHE = {}


def get_nc():
    if "nc" not in _NC_CACHE:
        nc = bacc.Bacc(None, target_bir_lowering=False)
        build_kernel(nc)
        nc.compile()
        _NC_CACHE["nc"] = nc
    return _NC_CACHE["nc"]


def host_consts(Wq, Wk, Wv, Wo, rel_emb):
    """Host-precomputed constant tensors (fp32 numpy)."""
    idx = np.clip(np.arange(-20, 21), -S + 1, S - 1) + 20
    pos = np.asarray(rel_emb, np.float32)[idx]          # (41,41) identity gather
    Wq = np.asarray(Wq, np.float32)
    Wk = np.asarray(Wk, np.float32)
    Wv = np.asarray(Wv, np.float32)
    Wo = np.asarray(Wo, np.float32)
    mcat = np.zeros((DIN, NP * 128), np.float32)
    gcat = np.zeros((NP * 128, U), np.float32)
    for h in range(H):
        M_h = (Wq[:, h * DH:(h + 1) * DH] @ Wk[:, h * DH:(h + 1) * DH].T
               / math.sqrt(DH))
        G_h = Wv[:, h * DH:(h + 1) * DH] @ Wo[h * DH:(h + 1) * DH, :]
        p, j = divmod(h, 2)
        mcat[:, p * 128 + j * SLOT: p * 128 + j * SLOT + S] = M_h
        gcat[p * 128 + j * SLOT: p * 128 + j * SLOT + S, :] = G_h
    pos_patT = np.tile(pos.T, (1, NB))                   # (41, 328)
    eye = np.eye(S, dtype=np.float32)
    posl = np.zeros((S, 128), np.float32)
    posl[:, 0:S] = eye
    posl[:, SLOT:KP] = eye
    onesdiag = np.zeros((KP, 128), np.float32)
    onesdiag[0:S, 0:SLOT] = 1.0
    onesdiag[SLOT:KP, SLOT:128] = 1.0
    identb = np.eye(128, dtype=np.float32)
    return mcat, gcat, pos_patT, posl, onesdiag, identb


def kernel(x, Wq, bq, Wk, bk, Wv, bv, Wo, bo, rel_emb):
    mcat, gcat, pos_patT, posl, onesdiag, identb = host_consts(
        Wq, Wk, Wv, Wo, rel_emb)

    import ml_dtypes
    bf = lambda a: np.asarray(a, np.float32).astype(ml_dtypes.bfloat16)
    x = bf(x)
    consts = {
        "mcat": bf(mcat), "gcat": bf(gcat), "pos_patT": bf(pos_patT),
        "posl": bf(posl), "onesdiag": bf(onesdiag), "identb": bf(identb),
    }
    nc = get_nc()
    in_maps = []
    for ci in range(NC):
        xi = x[ci * BC:(ci + 1) * BC].reshape(BC * S, DIN).copy()
        in_maps.append({"x": xi, **consts})
    res = run_bass_kernel_spmd(nc, in_maps, core_ids=list(range(NC)))
    out = np.concatenate(
        [np.asarray(res.results[i]["out"], np.float32).reshape(BC, S, U)
         for i in range(NC)], axis=0)
    return out
